# revision 6
# baseline (speedup 1.0000x reference)
"""ExternalAttention Trainium2 Bass kernel (bf16-I/O, transpose-free).

Math (per batch b, N = H*W = 4096 tokens, C = 512, K = 64):
    x      = inputs @ w1 + b1          [N, C]
    logits = x @ m0                    [N, K]
    attn   = softmax(logits, axis=N)
    y      = attn @ m1 @ w2            [N, C]
    out    = relu(BN_affine(y) + inputs)

Host-side folds (tiny C x C / C x K matrices, fp64):
    wm    = [w1 @ m0 | 0]                           [C, K+1]  (b1 @ m0 is a
            per-column softmax shift -> dropped; zero col makes exp emit a
            ones-row that carries the BN shift through mm2)
    scale = gamma / sqrt(bn_var + eps); shift = beta - bn_mean * scale
    w2m   = [m1 @ (w2 * scale) ; shift]             [K+1, C]
    => out = relu(colsoftmax(x_in @ wm) @ w2m + x_in)

Layout strategy: the host pre-transposes the input to xT [C, N] and converts
to bf16 (rel tol is 2e-2; bf16 end-to-end sims at 3.3e-3).  With channels on
partitions the kernel needs NO on-chip transposes:
    - mm1: logitsT[65, n-chunk] = sum_c4 wm[c4]^T @ xT[c4, chunk]  (PSUM)
    - ACT exp PSUM -> attn [65, N] bf16 with accumulated row sums
    - DVE folds the softmax 1/rowsum into w2m (per batch, [65, 512] — much
      cheaper than scaling attn [65, 4096]); ones-row stays unscaled
    - mm2 per (c4, 1024-token) tile: psum = w2m'^T @ attn  plus the residual
      injected by an identity matmul streaming xT (PE adds it in PSUM, no
      DVE pass); relu (ACT/DVE alternating) writes bf16 back over the xT
      tile in place; yT stored out in [C, N] layout, host un-transposes.
DMA: everything rides the sync (HWDGE) ring so loads drain strictly before
stores contend; input loads are ramped (512KB first) so mm1 starts ~2us in.
Per core: 8.4MB in + 8.4MB out bf16 ~= 47us at the 358 GB/s HBM-per-core
limit — the roofline for this kernel; PE (mm1+mm2+inject = 98k cycles
~= 41us) hides underneath it.
"""

import os
import sys
from contextlib import ExitStack

import numpy as np
import ml_dtypes

for _p in ("/opt/trn_rl_repo", os.path.expanduser("~/.axon_site/_ro/trn_rl_repo")):
    if os.path.isdir(_p) and _p not in sys.path:
        sys.path.insert(0, _p)

import concourse.bass as bass
import concourse.mybir as mybir
import concourse.tile as tile
from concourse import bacc
from concourse.bass import ts
from concourse.bass_utils import run_bass_kernel_spmd

B, H, W, C, K = 16, 64, 64, 512, 64
N = H * W  # 4096 tokens
BN_EPS = 1e-3
NCORES = 8
BPC = B // NCORES  # batches per core = 2

F32 = mybir.dt.float32
BF16 = mybir.dt.bfloat16
BF16NP = ml_dtypes.bfloat16

NCHUNK = 8  # 512-token mm1 chunks
NSC = 4     # 1024-token mm2 super-chunks (sc) of 2 j-halves each

_cached_nc = None


def _build_nc() -> bass.Bass:
    nc = bacc.Bacc(None, target_bir_lowering=False, debug=False)
    xt = nc.dram_tensor("xt", [BPC, C, N], BF16, kind="ExternalInput")
    wm = nc.dram_tensor("wm", [C, K + 1], BF16, kind="ExternalInput")
    w2m = nc.dram_tensor("w2m", [K + 1, C], BF16, kind="ExternalInput")
    ident = nc.dram_tensor("ident", [128, 128], BF16, kind="ExternalInput")
    yt = nc.dram_tensor("yt", [BPC, C, N], BF16, kind="ExternalOutput")

    with tile.TileContext(nc) as tc, ExitStack() as ctx:
        const = ctx.enter_context(tc.tile_pool(name="const", bufs=1))
        at_pool = ctx.enter_context(tc.tile_pool(name="at", bufs=2))
        attn_pool = ctx.enter_context(tc.tile_pool(name="attn", bufs=2))
        small = ctx.enter_context(tc.tile_pool(name="small", bufs=8))

        ident_sb = const.tile([128, 128], BF16)
        wm_sb = const.tile([128, 4, K + 1], BF16)   # [p, c4, k] = wm[c4*128+p, k]
        w2m_sb = const.tile([K + 1, C], BF16)

        # channel-major views: row c = c4*128 + p; tokens (sc, j, 512)
        xvs = [xt[b].rearrange("(c4 p) (sc j n) -> p c4 sc j n", p=128, j=2, n=512)
               for b in range(BPC)]
        yvs = [yt[b].rearrange("(c4 p) (sc j n) -> p c4 sc j n", p=128, j=2, n=512)
               for b in range(BPC)]

        ats, attns, sums_t, w2mbs = [], [], [], []
        for b in range(BPC):
            ats.append(at_pool.tile([128, 4, NSC, 2, 512], BF16, tag="at",
                                    name=f"at{b}"))
            attns.append(attn_pool.tile([K + 1, N], BF16, tag="attn",
                                        name=f"attn{b}"))
            sums_t.append(small.tile([K + 1, NCHUNK], F32, tag="sums",
                                     name=f"sums{b}"))
            w2mbs.append(small.tile([K + 1, C], BF16, tag="w2mb",
                                    name=f"w2mb{b}"))

        # ---- loads, all on the sync (HWDGE) ring, in priority order ----
        nc.sync.dma_start(out=wm_sb,
                          in_=wm.rearrange("(c4 p) k -> p c4 k", p=128))
        # b0 ramped: 512KB, 512KB, then 1MB x3 so mm1 starts ~2us in
        nc.sync.dma_start(out=ats[0][:, :, 0, 0], in_=xvs[0][:, :, 0, 0])
        nc.sync.dma_start(out=ats[0][:, :, 0, 1], in_=xvs[0][:, :, 0, 1])
        nc.sync.dma_start(out=ident_sb, in_=ident[:, :])
        nc.sync.dma_start(out=w2m_sb, in_=w2m[:, :])
        for sc in range(1, NSC):
            nc.sync.dma_start(out=ats[0][:, :, sc], in_=xvs[0][:, :, sc])
        # The 8 DMA queue lanes drain round-robin, so without ordering b1's
        # loads steal bandwidth from b0's still-pending data and push the
        # whole b0 chain (softmax -> mm2 b0) out by several us.  Gate b1's
        # loads on b0's last load landing: this tiny SBUF->SBUF copy reads
        # the tail of b0's sc3 slice (RAW on the b0 load) and writes a corner
        # of b1's sc0 target (WAW forces b1's first load behind it).
        nc.sync.dma_start(out=ats[1][0:1, 0, 0, 0, 0:2],
                          in_=ats[0][0:1, 3, 3, 1, 510:512])
        for sc in range(NSC):
            nc.sync.dma_start(out=ats[1][:, :, sc], in_=xvs[1][:, :, sc])

        partials = [small.tile([K + 1, 2], F32, tag="part", name=f"part{b}")
                    for b in range(BPC)]

        def mm1_chunk(b, q):
            """mm1 + exp for one 512-token chunk; pipelined partial row-sum
            reduction so the softmax tail after the last chunk is short."""
            at, attn, sums = ats[b], attns[b], sums_t[b]
            sc, j = divmod(q, 2)
            p_l = l_pool.tile([K + 1, 512], F32, tag="l")
            for c4 in range(4):
                nc.tensor.matmul(
                    p_l,
                    lhsT=wm_sb[:, c4],
                    rhs=at[:, c4, sc, j],
                    start=(c4 == 0),
                    stop=(c4 == 3),
                )
            nc.scalar.activation(
                out=attn[:, ts(q, 512)], in_=p_l,
                func=mybir.ActivationFunctionType.Exp,
                accum_out=sums[:, q:q + 1],
            )
            if q == 3:
                nc.vector.reduce_sum(out=partials[b][:, 0:1], in_=sums[:, 0:4],
                                     axis=mybir.AxisListType.X)

        def softmax_finish(b):
            """Fold 1/rowsum into per-batch w2m'; ones-row (k=K, the BN
            shift) stays unscaled.  c4=0 columns are produced first so mm2
            can load its first weights before the rest are scaled."""
            sums, w2mb = sums_t[b], w2mbs[b]
            rcp = small.tile([K + 1, 1], F32, tag="rcp", name=f"rcp{b}")
            nc.vector.reduce_sum(out=partials[b][:, 1:2], in_=sums[:, 4:8],
                                 axis=mybir.AxisListType.X)
            nc.vector.tensor_add(partials[b][:, 0:1], partials[b][:, 0:1],
                                 partials[b][:, 1:2])
            nc.vector.reciprocal(out=rcp, in_=partials[b][:, 0:1])
            nc.vector.tensor_scalar_mul(w2mb[0:K, 0:128], w2m_sb[0:K, 0:128],
                                        rcp[0:K])
            nc.vector.tensor_copy(w2mb[K:K + 1, 0:128], w2m_sb[K:K + 1, 0:128])
            nc.vector.tensor_scalar_mul(w2mb[0:K, 128:C], w2m_sb[0:K, 128:C],
                                        rcp[0:K])
            nc.vector.tensor_copy(w2mb[K:K + 1, 128:C], w2m_sb[K:K + 1, 128:C])

        def mm2_sc(b, i):
            """One (c4, sc) super-tile: w2m' matmul + residual inject + relu
            (ACT/DVE alternating) written bf16 in place over xT, store per
            half-row."""
            at, attn, w2mb = ats[b], attns[b], w2mbs[b]
            c4, sc = divmod(i, NSC)
            p_y = y_pool.tile([128, 2, 512], F32, tag="y")
            # residual injects first: they depend only on the input tile, so
            # the PE has buffered work while the softmax chain produces w2m';
            # grouping also halves the LDWEIGHTS swaps (ident, then w2m')
            for j in range(2):
                nc.tensor.matmul(
                    p_y[:, j],
                    lhsT=ident_sb,
                    rhs=at[:, c4, sc, j],
                    start=True, stop=False,
                )
            for j in range(2):
                nc.tensor.matmul(
                    p_y[:, j],
                    lhsT=w2mb[:, ts(c4, 128)],
                    rhs=attn[:, ts(2 * sc + j, 512)],
                    start=False, stop=True,
                )
            if (c4 + sc) % 2 == 0:
                nc.scalar.activation(
                    out=at[:, c4, sc], in_=p_y,
                    func=mybir.ActivationFunctionType.Relu,
                )
            else:
                nc.vector.tensor_scalar_max(at[:, c4, sc], p_y, 0.0)
            if sc % 2 == 1:
                nc.sync.dma_start(
                    out=yvs[b][:, c4, sc - 1:sc + 1],
                    in_=at[:, c4, sc - 1:sc + 1],
                )

        with tc.tile_pool(name="lps", bufs=2, space="PSUM") as l_pool, \
             tc.tile_pool(name="yps", bufs=3, space="PSUM") as y_pool:
            # PE warmup: HAM starts the tensor engine clock-gated at K=4/8
            # and only opens to 8/8 after ~10us of sustained activity.  Burn
            # scratch matmuls (no DMA deps -> dispatch at engine start) so
            # the ramp happens before the real work arrives.
            scr = const.tile([128, 512], BF16)
            nc.vector.memset(scr, 0.0)
            p_w = l_pool.tile([K + 1, 512], F32, tag="l", name="warm")
            for _ in range(16):
                nc.tensor.matmul(p_w, lhsT=scr[:, 0:K + 1], rhs=scr,
                                 start=True, stop=True)
            nc.vector.tensor_copy(partials[0][:, 0:1], p_w[:, 0:1])

            for q in range(NCHUNK):
                mm1_chunk(0, q)
            softmax_finish(0)
            # mm2 b0 with mm1 b1 chunks woven into the back half of the PE
            # stream — by then b1's (later-queued) loads have all landed, so
            # the in-order PE queue never stalls on them; b1's softmax chain
            # resolves while b0's epilogue drains
            for i in range(16):
                mm2_sc(0, i)
                if i >= 8:
                    mm1_chunk(1, i - 8)
            softmax_finish(1)
            for i in range(16):
                mm2_sc(1, i)

    nc.finalize()
    return nc


def _get_nc() -> bass.Bass:
    global _cached_nc
    if _cached_nc is None:
        _cached_nc = _build_nc()
    return _cached_nc


def _fold_weights(w1, m0, m1, w2, gamma, beta, bn_mean, bn_var):
    w1 = np.asarray(w1, np.float64)
    m0 = np.asarray(m0, np.float64)
    m1 = np.asarray(m1, np.float64)
    w2 = np.asarray(w2, np.float64)
    gamma = np.asarray(gamma, np.float64)
    beta = np.asarray(beta, np.float64)
    bn_mean = np.asarray(bn_mean, np.float64)
    bn_var = np.asarray(bn_var, np.float64)

    wm_aug = np.zeros((C, K + 1), np.float64)
    wm_aug[:, :K] = w1 @ m0  # col K stays 0 -> ones row out of exp
    scale = gamma / np.sqrt(bn_var + BN_EPS)
    w2m_aug = np.zeros((K + 1, C), np.float64)
    w2m_aug[:K] = m1 @ (w2 * scale[None, :])
    w2m_aug[K] = beta - bn_mean * scale  # shift row
    return wm_aug.astype(BF16NP), w2m_aug.astype(BF16NP)


def _run(inputs_np: dict, trace: bool = False):
    nc = _get_nc()
    wm_aug, w2m_aug = _fold_weights(
        inputs_np["w1"], inputs_np["m0"], inputs_np["m1"], inputs_np["w2"],
        inputs_np["gamma"], inputs_np["beta"],
        inputs_np["bn_mean"], inputs_np["bn_var"],
    )
    eye = np.eye(128, dtype=BF16NP)
    flat = np.asarray(inputs_np["inputs"], np.float32).reshape(B, N, C)
    in_maps = []
    for i in range(NCORES):
        xt = np.ascontiguousarray(
            flat[i * BPC:(i + 1) * BPC].transpose(0, 2, 1)).astype(BF16NP)
        in_maps.append({"xt": xt, "wm": wm_aug, "w2m": w2m_aug, "ident": eye})
    res = run_bass_kernel_spmd(nc, in_maps, core_ids=list(range(NCORES)),
                               trace=trace)
    out = np.concatenate(
        [np.asarray(r["yt"]).astype(np.float32).transpose(0, 2, 1)
         for r in res.results], axis=0)
    return np.ascontiguousarray(out).reshape(B, H, W, C), res


def kernel(**inputs) -> np.ndarray:
    out, _ = _run(inputs, trace=False)
    return out


# revision 8
# speedup vs baseline: 1.0309x; 1.0309x over previous
"""ExternalAttention Trainium2 Bass kernel (bf16-I/O, transpose-free).

Math (per batch b, N = H*W = 4096 tokens, C = 512, K = 64):
    x      = inputs @ w1 + b1          [N, C]
    logits = x @ m0                    [N, K]
    attn   = softmax(logits, axis=N)
    y      = attn @ m1 @ w2            [N, C]
    out    = relu(BN_affine(y) + inputs)

Host-side folds (tiny C x C / C x K matrices, fp64):
    wm    = [w1 @ m0 | 0]                           [C, K+1]  (b1 @ m0 is a
            per-column softmax shift -> dropped; zero col makes exp emit a
            ones-row that carries the BN shift through mm2)
    scale = gamma / sqrt(bn_var + eps); shift = beta - bn_mean * scale
    w2m   = [m1 @ (w2 * scale) ; shift]             [K+1, C]
    => out = relu(colsoftmax(x_in @ wm) @ w2m + x_in)

Layout strategy: the host pre-transposes the input to xT [C, N] and converts
to bf16 (rel tol is 2e-2; bf16 end-to-end sims at 3.3e-3).  With channels on
partitions the kernel needs NO on-chip transposes:
    - mm1: logitsT[65, n-chunk] = sum_c4 wm[c4]^T @ xT[c4, chunk]  (PSUM)
    - ACT exp PSUM -> attn [65, N] bf16 with accumulated row sums
    - DVE folds the softmax 1/rowsum into w2m (per batch, [65, 512] — much
      cheaper than scaling attn [65, 4096]); ones-row stays unscaled
    - mm2 per (c4, 1024-token) tile: psum = w2m'^T @ attn  plus the residual
      injected by an identity matmul streaming xT (PE adds it in PSUM, no
      DVE pass); relu (ACT/DVE alternating) writes bf16 back over the xT
      tile in place; yT stored out in [C, N] layout, host un-transposes.
DMA: everything rides the sync (HWDGE) ring so loads drain strictly before
stores contend; input loads are ramped (512KB first) so mm1 starts ~2us in.
Per core: 8.4MB in + 8.4MB out bf16 ~= 47us at the 358 GB/s HBM-per-core
limit — the roofline for this kernel; PE (mm1+mm2+inject = 98k cycles
~= 41us) hides underneath it.
"""

import os
import sys
from contextlib import ExitStack

import numpy as np
import ml_dtypes

for _p in ("/opt/trn_rl_repo", os.path.expanduser("~/.axon_site/_ro/trn_rl_repo")):
    if os.path.isdir(_p) and _p not in sys.path:
        sys.path.insert(0, _p)

import concourse.bass as bass
import concourse.mybir as mybir
import concourse.tile as tile
from concourse import bacc
from concourse.bass import ts
from concourse.bass_utils import run_bass_kernel_spmd

B, H, W, C, K = 16, 64, 64, 512, 64
N = H * W  # 4096 tokens
BN_EPS = 1e-3
NCORES = 8
BPC = B // NCORES  # batches per core = 2

F32 = mybir.dt.float32
BF16 = mybir.dt.bfloat16
BF16NP = ml_dtypes.bfloat16

NCHUNK = 8  # 512-token mm1 chunks
NSC = 4     # 1024-token mm2 super-chunks (sc) of 2 j-halves each

_cached_nc = None


def _build_nc() -> bass.Bass:
    nc = bacc.Bacc(None, target_bir_lowering=False, debug=False)
    xt = nc.dram_tensor("xt", [BPC, C, N], BF16, kind="ExternalInput")
    wm = nc.dram_tensor("wm", [C, K + 1], BF16, kind="ExternalInput")
    w2m = nc.dram_tensor("w2m", [K + 1, C], BF16, kind="ExternalInput")
    ident = nc.dram_tensor("ident", [128, 128], BF16, kind="ExternalInput")
    yt = nc.dram_tensor("yt", [BPC, C, N], BF16, kind="ExternalOutput")

    with tile.TileContext(nc) as tc, ExitStack() as ctx:
        const = ctx.enter_context(tc.tile_pool(name="const", bufs=1))
        at_pool = ctx.enter_context(tc.tile_pool(name="at", bufs=2))
        attn_pool = ctx.enter_context(tc.tile_pool(name="attn", bufs=2))
        small = ctx.enter_context(tc.tile_pool(name="small", bufs=8))

        ident_sb = const.tile([128, 128], BF16)
        wm_sb = const.tile([128, 4, K + 1], BF16)   # [p, c4, k] = wm[c4*128+p, k]
        w2m_sb = const.tile([K + 1, C], BF16)

        # channel-major views: row c = c4*128 + p; tokens (sc, j, 512)
        xvs = [xt[b].rearrange("(c4 p) (sc j n) -> p c4 sc j n", p=128, j=2, n=512)
               for b in range(BPC)]
        yvs = [yt[b].rearrange("(c4 p) (sc j n) -> p c4 sc j n", p=128, j=2, n=512)
               for b in range(BPC)]

        ats, attns, sums_t, w2mbs = [], [], [], []
        for b in range(BPC):
            ats.append(at_pool.tile([128, 4, NSC, 2, 512], BF16, tag="at",
                                    name=f"at{b}"))
            attns.append(attn_pool.tile([K + 1, N], BF16, tag="attn",
                                        name=f"attn{b}"))
            sums_t.append(small.tile([K + 1, NCHUNK], F32, tag="sums",
                                     name=f"sums{b}"))
            w2mbs.append(small.tile([K + 1, C], BF16, tag="w2mb",
                                    name=f"w2mb{b}"))

        # ---- loads, all on the sync (HWDGE) ring; one HW queue -> strict
        # FIFO delivery, so order = need-order.  b0's x is the serial prefix
        # that gates softmax(0) -> everything; consts other than wm (needed
        # by mm1) ride after it, b1 behind those.
        nc.sync.dma_start(out=wm_sb,
                          in_=wm.rearrange("(c4 p) k -> p c4 k", p=128))
        nc.sync.dma_start(out=ats[0][:, :, 0, 0], in_=xvs[0][:, :, 0, 0])
        nc.sync.dma_start(out=ats[0][:, :, 0, 1], in_=xvs[0][:, :, 0, 1])
        for sc in range(1, NSC - 1):
            nc.sync.dma_start(out=ats[0][:, :, sc], in_=xvs[0][:, :, sc])
        # last super-chunk split in two so the final chunk's sem fires early
        nc.sync.dma_start(out=ats[0][:, :, NSC - 1, 0], in_=xvs[0][:, :, NSC - 1, 0])
        nc.sync.dma_start(out=ats[0][:, :, NSC - 1, 1], in_=xvs[0][:, :, NSC - 1, 1])
        nc.sync.dma_start(out=ident_sb, in_=ident[:, :])
        nc.sync.dma_start(out=w2m_sb, in_=w2m[:, :])
        for sc in range(NSC):
            nc.sync.dma_start(out=ats[1][:, :, sc], in_=xvs[1][:, :, sc])

        partials = [small.tile([K + 1, 2], F32, tag="part", name=f"part{b}")
                    for b in range(BPC)]

        def mm1_chunk(b, q):
            """mm1 + exp for one 512-token chunk; pipelined partial row-sum
            reduction so the softmax tail after the last chunk is short."""
            at, attn, sums = ats[b], attns[b], sums_t[b]
            sc, j = divmod(q, 2)
            p_l = l_pool.tile([K + 1, 512], F32, tag="l")
            for c4 in range(4):
                nc.tensor.matmul(
                    p_l,
                    lhsT=wm_sb[:, c4],
                    rhs=at[:, c4, sc, j],
                    start=(c4 == 0),
                    stop=(c4 == 3),
                )
            nc.scalar.activation(
                out=attn[:, ts(q, 512)], in_=p_l,
                func=mybir.ActivationFunctionType.Exp,
                accum_out=sums[:, q:q + 1],
            )
            if q == 3:
                nc.vector.reduce_sum(out=partials[b][:, 0:1], in_=sums[:, 0:4],
                                     axis=mybir.AxisListType.X)

        def softmax_finish(b):
            """Fold 1/rowsum into per-batch w2m'; ones-row (k=K, the BN
            shift) stays unscaled.  c4=0 columns are produced first so mm2
            can load its first weights before the rest are scaled."""
            sums, w2mb = sums_t[b], w2mbs[b]
            rcp = small.tile([K + 1, 1], F32, tag="rcp", name=f"rcp{b}")
            nc.vector.reduce_sum(out=partials[b][:, 1:2], in_=sums[:, 4:8],
                                 axis=mybir.AxisListType.X)
            nc.vector.tensor_add(partials[b][:, 0:1], partials[b][:, 0:1],
                                 partials[b][:, 1:2])
            nc.vector.reciprocal(out=rcp, in_=partials[b][:, 0:1])
            nc.vector.tensor_scalar_mul(w2mb[0:K, 0:128], w2m_sb[0:K, 0:128],
                                        rcp[0:K])
            nc.vector.tensor_copy(w2mb[K:K + 1, 0:128], w2m_sb[K:K + 1, 0:128])
            nc.vector.tensor_scalar_mul(w2mb[0:K, 128:C], w2m_sb[0:K, 128:C],
                                        rcp[0:K])
            nc.vector.tensor_copy(w2mb[K:K + 1, 128:C], w2m_sb[K:K + 1, 128:C])

        def mm2_sc(b, i):
            """One (c4, sc) super-tile: w2m' matmul + residual inject + relu
            (ACT/DVE alternating) written bf16 in place over xT, store per
            half-row."""
            at, attn, w2mb = ats[b], attns[b], w2mbs[b]
            c4, sc = divmod(i, NSC)
            p_y = y_pool.tile([128, 2, 512], F32, tag="y")
            # residual injects first: they depend only on the input tile, so
            # the PE has buffered work while the softmax chain produces w2m';
            # grouping also halves the LDWEIGHTS swaps (ident, then w2m')
            for j in range(2):
                nc.tensor.matmul(
                    p_y[:, j],
                    lhsT=ident_sb,
                    rhs=at[:, c4, sc, j],
                    start=True, stop=False,
                )
            for j in range(2):
                nc.tensor.matmul(
                    p_y[:, j],
                    lhsT=w2mb[:, ts(c4, 128)],
                    rhs=attn[:, ts(2 * sc + j, 512)],
                    start=False, stop=True,
                )
            last = (b == BPC - 1 and i == 4 * NSC - 1)
            if last:
                # final tile: split relu across DVE+ACT and store in small
                # pieces so the kernel tail past the last matmul is short
                nc.vector.tensor_scalar_max(at[:, c4, sc, 0], p_y[:, 0], 0.0)
                nc.sync.dma_start(out=yvs[b][:, c4, sc - 1:sc], in_=at[:, c4, sc - 1:sc])
                nc.scalar.activation(
                    out=at[:, c4, sc, 1], in_=p_y[:, 1],
                    func=mybir.ActivationFunctionType.Relu,
                )
                nc.sync.dma_start(out=yvs[b][:, c4, sc:sc + 1, 0], in_=at[:, c4, sc:sc + 1, 0])
                nc.sync.dma_start(out=yvs[b][:, c4, sc:sc + 1, 1], in_=at[:, c4, sc:sc + 1, 1])
                return
            if (c4 + sc) % 2 == 0:
                nc.scalar.activation(
                    out=at[:, c4, sc], in_=p_y,
                    func=mybir.ActivationFunctionType.Relu,
                )
            else:
                nc.vector.tensor_scalar_max(at[:, c4, sc], p_y, 0.0)
            if sc % 2 == 1:
                nc.sync.dma_start(
                    out=yvs[b][:, c4, sc - 1:sc + 1],
                    in_=at[:, c4, sc - 1:sc + 1],
                )

        with tc.tile_pool(name="lps", bufs=2, space="PSUM") as l_pool, \
             tc.tile_pool(name="yps", bufs=3, space="PSUM") as y_pool:
            # PE warmup: HAM starts the tensor engine clock-gated at K=4/8
            # and only opens to 8/8 after ~10us of sustained activity.  Burn
            # scratch matmuls (no DMA deps -> dispatch at engine start) so
            # the ramp happens before the real work arrives.
            scr = const.tile([128, 512], BF16)
            nc.vector.memset(scr, 0.0)
            p_w = l_pool.tile([K + 1, 512], F32, tag="l", name="warm")
            for _ in range(16):
                nc.tensor.matmul(p_w, lhsT=scr[:, 0:K + 1], rhs=scr,
                                 start=True, stop=True)
            nc.vector.tensor_copy(partials[0][:, 0:1], p_w[:, 0:1])

            for q in range(NCHUNK):
                mm1_chunk(0, q)
            softmax_finish(0)
            # mm2 b0 with mm1 b1 chunks woven into the back half of the PE
            # stream — by then b1's (later-queued) loads have all landed, so
            # the in-order PE queue never stalls on them; b1's softmax chain
            # resolves while b0's epilogue drains
            for i in range(16):
                mm2_sc(0, i)
                if i >= 8:
                    mm1_chunk(1, i - 8)
            softmax_finish(1)
            for i in range(16):
                mm2_sc(1, i)

    nc.finalize()
    return nc


def _get_nc() -> bass.Bass:
    global _cached_nc
    if _cached_nc is None:
        _cached_nc = _build_nc()
    return _cached_nc


def _fold_weights(w1, m0, m1, w2, gamma, beta, bn_mean, bn_var):
    w1 = np.asarray(w1, np.float64)
    m0 = np.asarray(m0, np.float64)
    m1 = np.asarray(m1, np.float64)
    w2 = np.asarray(w2, np.float64)
    gamma = np.asarray(gamma, np.float64)
    beta = np.asarray(beta, np.float64)
    bn_mean = np.asarray(bn_mean, np.float64)
    bn_var = np.asarray(bn_var, np.float64)

    wm_aug = np.zeros((C, K + 1), np.float64)
    wm_aug[:, :K] = w1 @ m0  # col K stays 0 -> ones row out of exp
    scale = gamma / np.sqrt(bn_var + BN_EPS)
    w2m_aug = np.zeros((K + 1, C), np.float64)
    w2m_aug[:K] = m1 @ (w2 * scale[None, :])
    w2m_aug[K] = beta - bn_mean * scale  # shift row
    return wm_aug.astype(BF16NP), w2m_aug.astype(BF16NP)


def _run(inputs_np: dict, trace: bool = False):
    nc = _get_nc()
    wm_aug, w2m_aug = _fold_weights(
        inputs_np["w1"], inputs_np["m0"], inputs_np["m1"], inputs_np["w2"],
        inputs_np["gamma"], inputs_np["beta"],
        inputs_np["bn_mean"], inputs_np["bn_var"],
    )
    eye = np.eye(128, dtype=BF16NP)
    flat = np.asarray(inputs_np["inputs"], np.float32).reshape(B, N, C)
    in_maps = []
    for i in range(NCORES):
        xt = np.ascontiguousarray(
            flat[i * BPC:(i + 1) * BPC].transpose(0, 2, 1)).astype(BF16NP)
        in_maps.append({"xt": xt, "wm": wm_aug, "w2m": w2m_aug, "ident": eye})
    res = run_bass_kernel_spmd(nc, in_maps, core_ids=list(range(NCORES)),
                               trace=trace)
    out = np.concatenate(
        [np.asarray(r["yt"]).astype(np.float32).transpose(0, 2, 1)
         for r in res.results], axis=0)
    return np.ascontiguousarray(out).reshape(B, H, W, C), res


def kernel(**inputs) -> np.ndarray:
    out, _ = _run(inputs, trace=False)
    return out
